# revision 1
# baseline (speedup 1.0000x reference)
"""BoT block (conv1x1+BN+ReLU -> 4-head MHSA+posemb -> conv1x1+BN -> residual+ReLU)
on 8 trn2 NeuronCores, data-parallel over batch (2 images per core).

Self-contained: hardcodes shapes N=16, Cin=2048, H=W=32, heads=4, dqk=dv=128.
"""
import numpy as np

import concourse.bass as bass
import concourse.mybir as mybir
import concourse.tile as tile
from concourse import bacc
from concourse.bass_utils import run_bass_kernel_spmd
from concourse.masks import make_identity

EPS = 1e-5
HEADS = 4
DQK = 128
DV = 128
SCALE = DQK ** -0.5
N_IMG = 16
CIN = 2048
H = W = 32
HW = H * W            # 1024
MID = HEADS * DV      # 512
NCORES = 8
IMGS_PER_CORE = N_IMG // NCORES  # 2

P = 128
F16 = mybir.dt.float16
F32 = mybir.dt.float32
AF = mybir.ActivationFunctionType
ALU = mybir.AluOpType

KT1 = CIN // P        # 16 k-tiles for conv1
OT1 = MID // P        # 4 out-tiles for conv1
KT2 = MID // P        # 4 k-tiles for qk/v/conv3
OT3 = CIN // P        # 16 out-tiles for conv3
YT = HW // P          # 8 y-tiles
XT = HW // P          # 8 x-tiles
NH = HW // 512        # 2 halves of 512

_BUILT = {}


def _build():
    if "nc" in _BUILT:
        return _BUILT["nc"]
    nc = bacc.Bacc("TRN2", target_bir_lowering=False, debug=False,
                   num_devices=NCORES)

    # ---- DRAM I/O (per-core shard) ----
    xh_d = nc.dram_tensor("xh", [IMGS_PER_CORE, KT1, P, HW], F16, kind="ExternalInput")
    w1t_d = nc.dram_tensor("w1t", [KT1, P, MID], F16, kind="ExternalInput")
    qkwt_d = nc.dram_tensor("qkwt", [KT2, P, 2 * MID], F16, kind="ExternalInput")
    vwt_d = nc.dram_tensor("vwt", [KT2, P, MID], F16, kind="ExternalInput")
    w3t_d = nc.dram_tensor("w3t", [KT2, P, CIN], F16, kind="ExternalInput")
    embt_d = nc.dram_tensor("embt", [P, HW], F32, kind="ExternalInput")
    bn1_d = nc.dram_tensor("bn1", [P, 2, OT1], F32, kind="ExternalInput")
    add3_d = nc.dram_tensor("add3", [P, OT3], F32, kind="ExternalInput")
    out_d = nc.dram_tensor("out", [IMGS_PER_CORE, OT3, P, HW], F32,
                           kind="ExternalOutput")

    with tile.TileContext(nc) as tc:
        with (
            tc.tile_pool(name="consts", bufs=1) as consts,
            tc.tile_pool(name="xpool", bufs=18) as xpool,
            tc.tile_pool(name="feat", bufs=1) as featp,
            tc.tile_pool(name="qk", bufs=2) as qkp,
            tc.tile_pool(name="vaug", bufs=1) as vaugp,
            tc.tile_pool(name="et", bufs=2) as etp,
            tc.tile_pool(name="atf", bufs=2) as atfp,
            tc.tile_pool(name="small", bufs=6) as smallp,
            tc.tile_pool(name="xres", bufs=16) as xresp,
            tc.tile_pool(name="outp", bufs=2) as outp,
            tc.tile_pool(name="ps_mm", bufs=3, space="PSUM") as ps_mm,
            tc.tile_pool(name="ps_l", bufs=2, space="PSUM") as ps_l,
            tc.tile_pool(name="ps_at", bufs=2, space="PSUM") as ps_at,
            tc.tile_pool(name="ps_tr", bufs=1, space="PSUM") as ps_tr,
        ):
            # ---- conv1-critical constants (w1t interleaved with the x-tile
            # loads below so the first matmuls can start ASAP) ----
            w1t = consts.tile([P, KT1, MID], F16)
            bn1 = consts.tile([P, 2, OT1], F32)
            nc.sync.dma_start(bn1[:], bn1_d.ap())
            # remaining constants: DMAs deferred into conv1 of image 0
            qkwt = consts.tile([P, KT2, 2 * MID], F16)
            vwt = consts.tile([P, KT2, MID], F16)
            w3t = consts.tile([P, KT2, CIN], F16)
            embt = consts.tile([P, HW], F32)
            add3 = consts.tile([P, OT3], F32)
            ident = consts.tile([P, P], F16)

            for i in range(IMGS_PER_CORE):
                # ---- conv1 + BN + ReLU -> feat [c=512, hw] f16 ----
                # x streamed as [128,512] half-tiles, resident per nh-half.
                feat = featp.tile([P, KT2, HW], F16, tag="feat")
                with nc.named_scope(f"conv1_{i}"):
                    for nh in range(NH):
                        sl = slice(nh * 512, (nh + 1) * 512)
                        x_tiles = []
                        for k in range(KT1):
                            x_sb = xpool.tile([P, 512], F16, tag="x",
                                              name=f"x_{i}_{nh}_{k}")
                            nc.sync.dma_start(x_sb[:], xh_d.ap()[i, k, :, sl])
                            if i == 0 and nh == 0:
                                nc.sync.dma_start(w1t[:, k, :], w1t_d.ap()[k])
                            x_tiles.append(x_sb)
                        if i == 0 and nh == 1:
                            # deferred consts: loaded while conv1 img0 runs
                            for k in range(KT2):
                                nc.sync.dma_start(qkwt[:, k, :], qkwt_d.ap()[k])
                            for k in range(KT2):
                                nc.sync.dma_start(vwt[:, k, :], vwt_d.ap()[k])
                            for k in range(KT2):
                                nc.sync.dma_start(w3t[:, k, :], w3t_d.ap()[k])
                            nc.sync.dma_start(embt[:], embt_d.ap())
                            nc.sync.dma_start(add3[:], add3_d.ap())
                            make_identity(nc, ident[:])
                        for ot in range(OT1):
                            ps = ps_mm.tile([P, 512], F32, tag="mm")
                            for k in range(KT1):
                                nc.tensor.matmul(
                                    ps[:],
                                    w1t[:, k, ot * P:(ot + 1) * P],
                                    x_tiles[k][:],
                                    start=(k == 0), stop=(k == KT1 - 1),
                                )
                            nc.scalar.activation(
                                feat[:, ot, sl], ps[:], AF.Relu,
                                scale=bn1[:, 0, ot:ot + 1],
                                bias=bn1[:, 1, ot:ot + 1],
                            )

                # ---- qk: q (scaled) and k (+embT) [d, hw] f16 per head ----
                q_sb = qkp.tile([P, HEADS, HW], F16, tag="q")
                k_sb = qkp.tile([P, HEADS, HW], F16, tag="k")
                with nc.named_scope(f"qk_{i}"):
                    for ot in range(2 * HEADS):
                        for nh in range(NH):
                            ps = ps_mm.tile([P, 512], F32, tag="mm")
                            for k in range(KT2):
                                nc.tensor.matmul(
                                    ps[:],
                                    qkwt[:, k, ot * P:(ot + 1) * P],
                                    feat[:, k, nh * 512:(nh + 1) * 512],
                                    start=(k == 0), stop=(k == KT2 - 1),
                                )
                            sl = slice(nh * 512, (nh + 1) * 512)
                            if ot < HEADS:
                                nc.vector.tensor_copy(q_sb[:, ot, sl], ps[:])
                            else:
                                nc.vector.tensor_tensor(
                                    k_sb[:, ot - HEADS, sl], ps[:], embt[:, sl],
                                    ALU.add)

                # ---- vT (+ones col): [y, (h, d|1)] f16 ----
                vaug = vaugp.tile([P, YT, HEADS, 132], F16, tag="vaug")
                with nc.named_scope(f"v_{i}"):
                    nc.vector.memset(vaug[:, :, :, 128:129], 1.0)
                    for yt in range(YT):
                        ps = ps_mm.tile([P, 512], F32, tag="mm")
                        for k in range(KT2):
                            nc.tensor.matmul(
                                ps[:],
                                feat[:, k, yt * P:(yt + 1) * P],
                                vwt[:, k, :],
                                start=(k == 0), stop=(k == KT2 - 1),
                            )
                        nc.vector.tensor_copy(
                            vaug[:, yt, :, 0:128],
                            ps[:].rearrange("p (h d) -> p h d", d=128))

                # ---- prefetch ALL residual x tiles for conv3 up front:
                # fully resident (bufs=16) so these DMAs never slot-wait and
                # the next image's conv1 loads behind them flow freely ----
                xres_tiles = []
                for ot in range(OT3):
                    xr_sb = xresp.tile([P, HW], F16, tag="xr",
                                       name=f"xr_{i}_{ot}")
                    nc.sync.dma_start(xr_sb[:], xh_d.ap()[i, ot])
                    xres_tiles.append(xr_sb)

                # ---- attention per head ----
                atf = atfp.tile([P, HEADS, HW], F16, tag="atf")
                for h in range(HEADS):
                    with nc.named_scope(f"attn_{i}_{h}"):
                        et = etp.tile([P, YT, HW], F16, tag="et")
                        for yj in range(YT):
                            for nh in range(NH):
                                psl = ps_l.tile([P, 512], F32, tag="l")
                                nc.tensor.matmul(
                                    psl[:],
                                    k_sb[:, h, yj * P:(yj + 1) * P],
                                    q_sb[:, h, nh * 512:(nh + 1) * 512],
                                    start=True, stop=True,
                                )
                                nc.scalar.activation(
                                    et[:, yj, nh * 512:(nh + 1) * 512], psl[:],
                                    AF.Exp)
                        for xt in range(XT):
                            psa = ps_at.tile([P, 132], F32, tag="at")
                            for yj in range(YT):
                                nc.tensor.matmul(
                                    psa[:, 0:129],
                                    et[:, yj, xt * P:(xt + 1) * P],
                                    vaug[:, yj, h, 0:129],
                                    start=(yj == 0), stop=(yj == YT - 1),
                                )
                            rc = smallp.tile([P, 1], F32, tag="rc")
                            nc.vector.reciprocal(rc[:], psa[:, 128:129])
                            a_sb = smallp.tile([P, P], F16, tag="asb")
                            nc.vector.tensor_scalar(
                                a_sb[:], psa[:, 0:128], rc[:], 0.0,
                                ALU.mult, ALU.max)
                            pst = ps_tr.tile([P, P], F16, tag="tr")
                            nc.tensor.transpose(pst[:], a_sb[:], ident[:])
                            nc.vector.tensor_copy(
                                atf[:, h, xt * P:(xt + 1) * P], pst[:])

                # ---- conv3 (+bn3 folded) + residual + ReLU ----
                with nc.named_scope(f"conv3_{i}"):
                    for ot in range(OT3):
                        o_sb = outp.tile([P, HW], F32, tag="o")
                        for nh in range(NH):
                            sl = slice(nh * 512, (nh + 1) * 512)
                            ps = ps_mm.tile([P, 512], F32, tag="mm")
                            for k in range(KT2):
                                nc.tensor.matmul(
                                    ps[:],
                                    w3t[:, k, ot * P:(ot + 1) * P],
                                    atf[:, k, nh * 512:(nh + 1) * 512],
                                    start=(k == 0), stop=False,
                                )
                            # residual folded in as a final identity matmul
                            nc.tensor.matmul(
                                ps[:], ident[:], xres_tiles[ot][:, sl],
                                start=False, stop=True)
                            if nh == 0:
                                nc.scalar.activation(
                                    o_sb[:, sl], ps[:], AF.Relu,
                                    bias=add3[:, ot:ot + 1])
                            else:
                                nc.vector.tensor_scalar(
                                    o_sb[:, sl], ps[:], add3[:, ot:ot + 1],
                                    0.0, ALU.add, ALU.max)
                        # store on the ACT hwdge queue: keeps the sync queue
                        # loads-only so x/xres loads are never stuck behind
                        # compute-gated stores
                        nc.scalar.dma_start(out_d.ap()[i, ot], o_sb[:])

    nc.compile()
    _BUILT["nc"] = nc
    return nc


def _prep_maps(x, conv1_w, gamma1, beta1, mean1, var1, qk_w, v_w, pos_h, pos_w,
               conv3_w, gamma3, beta3, mean3, var3):
    f16 = np.float16
    inv1 = (gamma1 / np.sqrt(var1 + EPS)).astype(np.float32)
    add1 = (beta1 - mean1 * inv1).astype(np.float32)
    inv3 = (gamma3 / np.sqrt(var3 + EPS)).astype(np.float32)
    add3 = (beta3 - mean3 * inv3).astype(np.float32)

    w1t = np.ascontiguousarray(conv1_w.T).reshape(KT1, P, MID).astype(f16)
    qk_mod = np.concatenate([qk_w[:HEADS * DQK] * SCALE, qk_w[HEADS * DQK:]], 0)
    qkwt = np.ascontiguousarray(qk_mod.T).reshape(KT2, P, 2 * MID).astype(f16)
    vwt = np.ascontiguousarray(v_w.T).reshape(KT2, P, MID).astype(f16)
    w3t = np.ascontiguousarray((conv3_w * inv3[:, None]).T).reshape(
        KT2, P, CIN).astype(f16)
    embt = np.ascontiguousarray(
        (pos_h[:, None, :] + pos_w[None, :, :]).reshape(HW, DQK).T
    ).astype(np.float32)
    bn1 = np.stack([inv1.reshape(OT1, P).T, add1.reshape(OT1, P).T], 1)
    bn1 = np.ascontiguousarray(bn1).astype(np.float32)      # [P, 2, OT1]
    add3_t = np.ascontiguousarray(add3.reshape(OT3, P).T).astype(np.float32)

    xh_all = x.reshape(N_IMG, CIN // P, P, HW).astype(f16)

    in_maps = []
    for c in range(NCORES):
        sl = slice(c * IMGS_PER_CORE, (c + 1) * IMGS_PER_CORE)
        in_maps.append({
            "xh": np.ascontiguousarray(xh_all[sl]),
            "w1t": w1t, "qkwt": qkwt, "vwt": vwt, "w3t": w3t,
            "embt": embt, "bn1": bn1, "add3": add3_t,
        })
    return in_maps


def kernel(**inputs):
    nc = _build()
    inputs = {k: np.asarray(v) for k, v in inputs.items()}
    in_maps = _prep_maps(**inputs)
    res = run_bass_kernel_spmd(nc, in_maps, core_ids=list(range(NCORES)))
    out = np.concatenate([r["out"] for r in res.results], 0)
    return out.reshape(N_IMG, CIN, H, W).astype(np.float32)

